# revision 2
# baseline (speedup 1.0000x reference)
"""Mixtral-style MoE (B=4, S=2048, H=2048, I=5632, E=8, top-2, integer softmax)
on 8 Trainium2 NeuronCores — Strassen level-1 on the SwiGLU up-projections.

Strategy: expert-parallel, host routing (exact), one expert per core over its
first 2048 gathered tokens (spill to host fp32 as in the baseline). The PE is
the bottleneck (baseline 98.5% busy, 95.7% MFU), so this kernel reduces PE
work with one level of Strassen on the two phase-A matmuls (w1/w3): 7/8 of
the classical multiply cycles. To afford the 1.75x weight-combo DMA volume,
the loop is restructured so each weight tile streams from DRAM exactly once:
the x-side Strassen combos for all 4 token groups stay resident in SBUF
(112KB/partition) while ii runs over the 22 row-block pairs; h spills to a
DRAM scratch in bf16 and phase B (w2, classical) reloads it per group.

Combine scheduling: the 7 Strassen products per (row-pair, group, path)
accumulate in 7 PSUM banks; ScalarE drains the 4 quadrant-initializing
copies, VectorE applies the 8 +/- merges, ScalarE applies silu, VectorE forms
h = silu(g)*u in bf16. PE ~ 12.2us per step vs DVE ~ 8.1us, ScalarE ~ 4.7us.
"""
import os
import sys

if "/opt/trn_rl_repo" not in sys.path:
    sys.path.insert(0, "/opt/trn_rl_repo")

import numpy as np
import ml_dtypes

import concourse.bacc as bacc
import concourse.mybir as mybir
from concourse import tile
from concourse import bass_utils

# problem shapes
B, S, H, I, E = 4, 2048, 2048, 5632, 8
T = B * S                      # 8192 tokens
TOP_K = 2
Q_IN, LUT_MIN, Q_OUT = 128, -1024, 1 << 16

P = 128
KT = H // P                    # 16 contraction tiles for H
IT = I // P                    # 44 i-tiles
HT = H // P                    # 16 output tiles

G = 4                          # token groups per core
W512 = 512                     # tokens per group
TH = 256                       # Strassen token half
KT2 = KT // 2                  # 8 contraction tiles per H-half
IT2 = IT // 2                  # 22 row-tile pairs

f32 = mybir.dt.float32
bf16 = mybir.dt.bfloat16
BF16 = ml_dtypes.bfloat16

_EXP_LUT_CACHE = None


def _exp_lut():
    """Q16 exp LUT, computed with jax exactly as the reference does."""
    global _EXP_LUT_CACHE
    if _EXP_LUT_CACHE is None:
        import jax.numpy as jnp
        _EXP_LUT_CACHE = np.asarray(
            (jnp.exp(jnp.arange(LUT_MIN, 1, dtype=jnp.float32) / Q_IN) * Q_OUT
             ).astype(jnp.int32)
        )
    return _EXP_LUT_CACHE


def _route(x2d, w_gate):
    """Exact replication of the reference integer-softmax top-2 routing."""
    lg = (x2d.astype(np.float64) @ w_gate.T.astype(np.float64)).astype(np.float32)
    li = np.rint(lg * np.float32(128.0)).astype(np.int32)
    shifted = np.clip(li - li.max(axis=-1, keepdims=True), LUT_MIN, None)
    ev = _exp_lut()[shifted - LUT_MIN]
    gt = ev[:, None, :] > ev[:, :, None]
    eq = ev[:, None, :] == ev[:, :, None]
    jlt = np.arange(E)[None, None, :] < np.arange(E)[None, :, None]
    cnt = (gt | (eq & jlt)).sum(-1)
    sel = cnt < TOP_K
    evf = ev.astype(np.float32)
    den = (evf * sel).sum(-1, keepdims=True)
    wts = np.where(sel, evf / den, np.float32(0.0)).astype(np.float32)
    return sel, wts


CAP = T * TOP_K // E           # 2048 per-core device capacity

_BUILD_CACHE = {}

ADD = mybir.AluOpType.add
SUB = mybir.AluOpType.subtract
MUL = mybir.AluOpType.mult
Silu = (mybir.ActivationFunctionType.Sigmoid
        if os.environ.get("KS_SIM_SIGMOID")
        else mybir.ActivationFunctionType.Silu)
Copy = mybir.ActivationFunctionType.Copy


def _build():
    """Bass program: one expert's SwiGLU FFN over C=2048 tokens.

    Phase A (Strassen): for each of 22 row-tile pairs ii (rows ii and 22+ii),
    stream w1/w3 Strassen combo tiles once, then for each token group g and
    each path (w1->g, w3->u) run 7 products M1..M7 (8 matmuls each, 256-wide)
    and combine into the two row tiles; silu+mult -> h (bf16) -> DRAM scratch.

    Phase B (classical): per group, reload h[g] (44KB/partition), stream w2,
    16 ht-tiles x 44 accumulating matmuls, combine-weight multiply, store yt.
    """
    if "nc" in _BUILD_CACHE:
        return _BUILD_CACHE["nc"]

    nc = bacc.Bacc("TRN2", target_bir_lowering=False, debug=False, num_devices=8)
    xs_d = nc.dram_tensor("xsp", [P, G * 7 * KT2 * TH], bf16, kind="ExternalInput").ap()
    w1s_d = nc.dram_tensor("w1s", [IT2, P, 7 * KT2 * P], bf16, kind="ExternalInput").ap()
    w3s_d = nc.dram_tensor("w3s", [IT2, P, 7 * KT2 * P], bf16, kind="ExternalInput").ap()
    w2_d = nc.dram_tensor("w2p", [HT, P, I], bf16, kind="ExternalInput").ap()
    wv_d = nc.dram_tensor("wv", [P, CAP], f32, kind="ExternalInput").ap()
    yt_d = nc.dram_tensor("yt", [H, CAP], f32, kind="ExternalOutput").ap()
    h_d = nc.dram_tensor("hscratch", [G, P, IT * W512], bf16, kind="Internal").ap()

    with tile.TileContext(nc) as tc:
        # ---------------- phase A ----------------
        # phase-B's first w2 tile, prefetched during phase A so the first
        # ht of phase B only waits on the h reload (pool outlives phase A)
        w2f_pool = tc.alloc_tile_pool(name="w2f", bufs=1)
        w2f_t = w2f_pool.tile([P, I], bf16, tag="w2f")

        with (
            tc.tile_pool(name="xs", bufs=1) as xs_pool,
            tc.tile_pool(name="w1s", bufs=2) as w1s_pool,
            tc.tile_pool(name="w3s", bufs=2) as w3s_pool,
            tc.tile_pool(name="cq", bufs=6) as cq_pool,
            tc.tile_pool(name="sg", bufs=2) as sg_pool,
            tc.tile_pool(name="hst", bufs=4) as hst_pool,
            tc.tile_pool(name="warm", bufs=1) as warm_pool,
            tc.tile_pool(name="ps", bufs=8, space="PSUM") as ps_pool,
        ):
            # HAM warmup: bridge the first-DMA cold window (~25us) so the PE
            # clock gate is at 8/8 when real work starts.
            warm_t = warm_pool.tile([P, 512], bf16, tag="warm")
            nc.gpsimd.memset(warm_t[:], 0.0)
            warm_ps = ps_pool.tile([P, 512], f32, tag="m", name="warm_ps")
            for _ in range(108):
                nc.tensor.matmul(
                    warm_ps[:], warm_t[:, :P], warm_t[:], start=True, stop=True
                )

            # resident x-side Strassen combos for all 4 groups (112KB/part)
            xs_t = xs_pool.tile([P, G * 7 * KT2 * TH], bf16, tag="xs")
            GSTRIDE = 7 * KT2 * TH
            for g in range(G):
                nc.scalar.dma_start(
                    xs_t[:, g * GSTRIDE:(g + 1) * GSTRIDE],
                    xs_d[:, g * GSTRIDE:(g + 1) * GSTRIDE],
                )
            # after the startup-critical xs loads; needed only at phase B
            nc.scalar.dma_start(w2f_t[:], w2_d[0, :, :])
            xv = xs_t[:].rearrange("p (g j kt t) -> p g j kt t", g=G, j=7, kt=KT2)

            for ii in range(IT2):
                w1s_t = w1s_pool.tile([P, 7 * KT2 * P], bf16, tag="w1s")
                nc.sync.dma_start(w1s_t[:], w1s_d[ii, :, :])
                w3s_t = w3s_pool.tile([P, 7 * KT2 * P], bf16, tag="w3s")
                nc.sync.dma_start(w3s_t[:], w3s_d[ii, :, :])
                w1v = w1s_t[:].rearrange("p (j kt i) -> p j kt i", j=7, kt=KT2)
                w3v = w3s_t[:].rearrange("p (j kt i) -> p j kt i", j=7, kt=KT2)

                for g in range(G):
                    cs = []  # (c_lo, c_hi) for g-path then u-path
                    for wv_ in (w1v, w3v):
                        ms = []
                        for j in range(7):
                            mj = ps_pool.tile([P, TH], f32, tag="m",
                                              name=f"m{j}")
                            for kt in range(KT2):
                                nc.tensor.matmul(
                                    mj[:], wv_[:, j, kt, :], xv[:, g, j, kt, :],
                                    start=(kt == 0), stop=(kt == KT2 - 1),
                                )
                            ms.append(mj)
                        c_lo = cq_pool.tile([P, W512], f32, tag="c", name="c_lo")
                        c_hi = cq_pool.tile([P, W512], f32, tag="c", name="c_hi")
                        lo0, lo1 = c_lo[:, 0:TH], c_lo[:, TH:W512]
                        hi0, hi1 = c_hi[:, 0:TH], c_hi[:, TH:W512]
                        # quadrant inits on ScalarE (PSUM->SBUF copies)
                        nc.scalar.activation(lo0, ms[0][:], Copy)   # C11 = M1
                        nc.scalar.activation(hi1, ms[0][:], Copy)   # C22 = M1
                        nc.scalar.activation(hi0, ms[1][:], Copy)   # C21 = M2
                        nc.scalar.activation(lo1, ms[2][:], Copy)   # C12 = M3
                        # +/- merges on VectorE
                        nc.vector.tensor_tensor(hi1, hi1, ms[1][:], op=SUB)  # -M2
                        nc.vector.tensor_tensor(hi1, hi1, ms[2][:], op=ADD)  # +M3
                        nc.vector.tensor_tensor(lo0, lo0, ms[3][:], op=ADD)  # +M4
                        nc.vector.tensor_tensor(hi0, hi0, ms[3][:], op=ADD)  # +M4
                        nc.vector.tensor_tensor(lo0, lo0, ms[4][:], op=SUB)  # -M5
                        nc.vector.tensor_tensor(lo1, lo1, ms[4][:], op=ADD)  # +M5
                        nc.vector.tensor_tensor(hi1, hi1, ms[5][:], op=ADD)  # +M6
                        nc.vector.tensor_tensor(lo0, lo0, ms[6][:], op=ADD)  # +M7
                        cs.append((c_lo, c_hi))
                    (g_lo, g_hi), (u_lo, u_hi) = cs
                    for c_g, c_u, it in ((g_lo, u_lo, ii), (g_hi, u_hi, IT2 + ii)):
                        sg_t = sg_pool.tile([P, W512], f32, tag="sg")
                        nc.scalar.activation(sg_t[:], c_g[:], Silu)
                        h_t = hst_pool.tile([P, W512], bf16, tag="h")
                        nc.vector.tensor_tensor(h_t[:], sg_t[:], c_u[:], op=MUL)
                        nc.gpsimd.dma_start(
                            h_d[g, :, it * W512:(it + 1) * W512], h_t[:]
                        )

        # ---------------- phase B ----------------
        with (
            tc.tile_pool(name="hin", bufs=8) as hin_pool,
            tc.tile_pool(name="w2", bufs=3) as w2_pool,
            tc.tile_pool(name="ysb", bufs=3) as ysb_pool,
            tc.tile_pool(name="wvp", bufs=4) as wv_pool,
            tc.tile_pool(name="psB", bufs=4, space="PSUM") as psB_pool,
        ):
            NCH = 4
            ITC = IT // NCH          # 11 it-tiles per chunk
            CH = ITC * W512

            # combine weights for all groups first (tiny, ahead of the bulky
            # h reloads on the same gpsimd queue)
            wv_ts = []
            for g in range(G):
                wv_t = wv_pool.tile([P, W512], f32, tag="wv", name=f"wv{g}")
                nc.gpsimd.dma_start(wv_t[:], wv_d[:, g * W512:(g + 1) * W512])
                wv_ts.append(wv_t)

            def load_hin(g):
                # separate sub-tiles per chunk so the first ht's matmuls only
                # wait for chunk 0. Rides the gpsimd queue: FIFO after the h
                # stores = correct RAW order on the DRAM scratch.
                chunks = []
                for c in range(NCH):
                    hc = hin_pool.tile([P, CH], bf16, tag="hin", name=f"hin{c}")
                    nc.gpsimd.dma_start(hc[:], h_d[g, :, c * CH:(c + 1) * CH])
                    chunks.append(hc[:].rearrange("p (it t) -> p it t", it=ITC))
                return chunks

            hv = load_hin(0)
            for g in range(G):
                ts = slice(g * W512, (g + 1) * W512)
                wv_t = wv_ts[g]
                # prefetch next group's h before this group's yt stores queue
                # up on gpsimd
                hv_next = load_hin(g + 1) if g < G - 1 else None
                for ht in range(HT):
                    if g == 0 and ht == 0:
                        w2_t = w2f_t
                    else:
                        w2_t = w2_pool.tile([P, I], bf16, tag="w2")
                        nc.scalar.dma_start(w2_t[:], w2_d[ht, :, :])
                    y_ps = psB_pool.tile([P, W512], f32, tag="y")
                    for it in range(IT):
                        nc.tensor.matmul(
                            y_ps[:], w2_t[:, it * P:(it + 1) * P],
                            hv[it // ITC][:, it % ITC, :],
                            start=(it == 0), stop=(it == IT - 1),
                        )
                    y_sb = ysb_pool.tile([P, W512], f32, tag="ysb")
                    nc.vector.tensor_tensor(y_sb[:], y_ps[:], wv_t[:], op=MUL)
                    yt_eng = (nc.sync if g == G - 1 and ht == HT - 1
                              else nc.gpsimd)
                    yt_eng.dma_start(yt_d[ht * P:(ht + 1) * P, ts], y_sb[:])
                hv = hv_next

        w2f_pool.release()

    nc.compile()
    _BUILD_CACHE["nc"] = nc
    return nc


def _prep_w13s(w):
    """Strassen stationary combos for one [I, H] weight stack [E, I, H] ->
    [E, IT2, P, 7*KT2*P] bf16 with layout [ii, p, j, kt, i2] =
    W_j[ii*128 + i2, kt*128 + p]."""
    ne = w.shape[0]
    I2, H2 = I // 2, H // 2
    A11 = w[:, :I2, :H2]
    A12 = w[:, :I2, H2:]
    A21 = w[:, I2:, :H2]
    A22 = w[:, I2:, H2:]
    combos = np.stack([
        A11 + A22,       # M1
        A21 + A22,       # M2
        A11,             # M3
        A22,             # M4
        A11 + A12,       # M5
        A21 - A11,       # M6
        A12 - A22,       # M7
    ], axis=1)           # [ne, 7, I2, H2]
    # [ne, 7, ii, i2, kt, p] -> [ne, ii, p, j, kt, i2]
    t = combos.reshape(ne, 7, IT2, P, KT2, P).transpose(0, 2, 5, 1, 4, 3)
    return np.ascontiguousarray(t).reshape(ne, IT2, P, 7 * KT2 * P).astype(BF16)


def _prep_xs(xg):
    """Strassen moving combos for one expert's gathered tokens.
    xg: [H, CAP] fp32 -> [P, G*7*KT2*TH] bf16 with layout [p, g, j, kt, t] =
    X_j[kt*128 + p, t]."""
    H2 = H // 2
    out = np.empty((P, G, 7, KT2, TH), np.float32)
    for g in range(G):
        blk = xg[:, g * W512:(g + 1) * W512]
        B11 = blk[:H2, 0:TH]
        B12 = blk[:H2, TH:W512]
        B21 = blk[H2:, 0:TH]
        B22 = blk[H2:, TH:W512]
        xj = np.stack([
            B11 + B22,   # M1
            B11,         # M2
            B12 - B22,   # M3
            B21 - B11,   # M4
            B22,         # M5
            B11 + B12,   # M6
            B21 + B22,   # M7
        ], axis=0)       # [7, H2, TH]
        out[:, g] = xj.reshape(7, KT2, P, TH).transpose(2, 0, 1, 3)
    return np.ascontiguousarray(out.reshape(P, G * 7 * KT2 * TH)).astype(BF16)


def _prep_w2(w2):
    """w2p[e][ht, p, it*128+hh] = w2[e][ht*128+hh, it*128+p]  ([HT, 128, I])."""
    ne = w2.shape[0]
    return np.ascontiguousarray(
        w2.reshape(ne, HT, P, IT, P).transpose(0, 1, 4, 3, 2)
    ).reshape(ne, HT, P, I).astype(BF16)


def kernel(x, w_gate, w1, w2, w3):
    x = np.asarray(x, dtype=np.float32)
    w_gate = np.asarray(w_gate, dtype=np.float32)
    w1 = np.asarray(w1, dtype=np.float32)
    w2 = np.asarray(w2, dtype=np.float32)
    w3 = np.asarray(w3, dtype=np.float32)

    x2d = x.reshape(T, H)
    trace = bool(int(os.environ.get("BASS_MOE_TRACE", "0")))

    sel, wts = _route(x2d, w_gate)

    C = CAP
    w1s = _prep_w13s(w1)
    w3s = _prep_w13s(w3)
    w2p = _prep_w2(w2)
    xb = np.ascontiguousarray(x2d.T)                 # [H, T] fp32

    spill, idxs, in_maps = [], [], []
    for e in range(E):
        idx = np.nonzero(sel[:, e])[0]
        if len(idx) > C:
            spill.append((e, idx[C:]))
            idx = idx[:C]
        idxs.append(idx)
        n = len(idx)
        xg = np.zeros((H, C), np.float32)
        xg[:, :n] = xb[:, idx]
        wv = np.zeros(C, np.float32)
        wv[:n] = wts[idx, e]
        in_maps.append({
            "xsp": _prep_xs(xg),
            "w1s": w1s[e],
            "w3s": w3s[e],
            "w2p": w2p[e],
            "wv": np.broadcast_to(wv, (P, C)).copy(),
        })

    nc = _build()
    res = bass_utils.run_bass_kernel_spmd(
        nc, in_maps, core_ids=list(range(8)), trace=trace
    )
    if trace:
        kernel.last_exec_time_ns = res.exec_time_ns

    out2d = np.zeros((T, H), np.float32)
    for e in range(E):
        idx = idxs[e]
        out2d[idx] += res.results[e]["yt"].T[:len(idx)]

    # host fp32 FFN for the capacity-overflow token/expert pairs
    for e, idx in spill:
        xs = x2d[idx]
        g = xs @ w1[e].T
        sig = np.where(g >= 0.0, 1.0 / (1.0 + np.exp(-np.abs(g))),
                       np.exp(-np.abs(g)) / (1.0 + np.exp(-np.abs(g))))
        hh = (g * sig) * (xs @ w3[e].T)
        out2d[idx] += wts[idx, e:e + 1] * (hh @ w2[e].T)
    return out2d.reshape(B, S, H)


kernel.last_exec_time_ns = None
